# revision 1
# baseline (speedup 1.0000x reference)
"""Multi-head causal attention block on 8 Trainium2 NeuronCores.

Sharding: tensor-parallel over heads (4 groups of 4 heads) x data-parallel
over batch (2). Core c -> (batch b=c//4, head-group g=c%4). Each core
computes q/k/v projections for its head group, causal attention for its 4
heads, and a partial output projection; the host sums the 4 partials per
batch. All layout transposes are done host-side so the device does none.

Self-contained: hardcodes shapes for the 2x2048x2048, 16-head problem.
"""

import os
from contextlib import ExitStack

import numpy as np

import concourse.bass as bass
import concourse.tile as tile
from concourse import bacc, mybir
from concourse.bass import ds, ts
from concourse.bass_utils import run_bass_kernel_spmd

F32 = mybir.dt.float32
F32R = mybir.dt.float32r
ACTF = mybir.ActivationFunctionType

# Full-problem dims
BATCH = 2
SEQ = 2048
D_MODEL = 2048
NUM_HEADS = 16
HEAD_DIM = 128
N_CORES = 8
N_GROUPS = 4  # head-groups (tensor parallel)
DG = D_MODEL // N_GROUPS  # 512 = 4 heads per group
SCALE = 1.0 / float(np.sqrt(HEAD_DIM))

QB = 512  # q-block width in attention
KT = 128  # k-tile width (partition dim)

USE_F32R = os.environ.get("KERNEL_F32", "0") != "1"
MMDT = F32R if USE_F32R else F32


def _r(ap):
    """View a float32 DRAM AP as the matmul dtype for DMA into MMDT tiles."""
    return ap.bitcast(F32R) if USE_F32R else ap


def _mha_body(ctx, tc, aps, S, D, DGl):
    """Per-core kernel body.

    aps: dict of DRAM APs: xt [D,S], wqt/wkt/wvt [D,DGl], wot [DGl,D],
      bq/bk [128, DGl//128], bv [128, DGl], bo [128, D], masks [4,128,QB],
      ones [128,1], out [S,D].

    k^T and v stay resident in SBUF (written directly by the projection
    drains); only q^T round-trips through DRAM.
    """
    nc = tc.nc
    n_kd = D // 128  # contraction tiles over d_model
    n_sq = S // QB  # 512-wide attention q-blocks
    n_sk = S // KT  # 128-wide seq tiles
    n_dg = DGl // 128  # head tiles per group
    QB1 = 256  # phase-1 seq-slice width
    n_ns = S // QB1

    xt, wqt, wkt, wvt, wot = aps["xt"], aps["wqt"], aps["wkt"], aps["wvt"], aps["wot"]
    out = aps["out"]

    # DRAM scratch for v [S, DGl] (q^T and k^T stay resident in SBUF)
    dram = ctx.enter_context(tc.tile_pool(name="dram", bufs=1, space="DRAM"))
    v_d = dram.tile([S, DGl], F32, name="v_d")

    consts = ctx.enter_context(tc.tile_pool(name="consts", bufs=1))
    # dummy activation first: forces the ACT function-table DMA to queue
    # ahead of the bulk input loads (else every early PSUM drain stalls)
    warm = consts.tile([128, 1], F32, name="act_warm")
    nc.vector.memset(warm[:], 0.0)
    nc.scalar.activation(warm[:], warm[:], ACTF.Identity, bias=warm[:, 0:1])
    ones_sb = consts.tile([128, 1], MMDT, name="ones_sb")
    bq_sb = consts.tile([128, n_dg], F32, name="bq_sb")
    bk_sb = consts.tile([128, n_dg], F32, name="bk_sb")
    bv_sb = consts.tile([128, DGl], F32, name="bv_sb")
    masks_sb = consts.tile([128, 4 * QB], F32, name="masks_sb")

    # resident q^T / k^T per head: [p, s] = q^T/k^T[h*128+p, s]
    kv_pool = ctx.enter_context(tc.tile_pool(name="kv_res", bufs=1))
    kt_res = [
        kv_pool.tile([128, S], MMDT, tag=f"ktr{h}", name=f"kt_res{h}")
        for h in range(n_dg)
    ]
    qt_res = [
        kv_pool.tile([128, S], MMDT, tag=f"qtr{h}", name=f"qt_res{h}")
        for h in range(n_dg)
    ]

    # ---------------- Phase 1: q/k/v projections ----------------
    with (
        tc.tile_pool(name="wqkv", bufs=1) as wpool,
        tc.tile_pool(name="xt_pool", bufs=2) as xpool,
        tc.tile_pool(name="p1_stage", bufs=2) as stage,
        tc.tile_pool(name="p1_psum", bufs=4, space="PSUM") as psum1,
    ):
        # weights resident: w*_sb[p, k*DGl + f] = w*t[k*128+p, f]
        w_sbs = {
            wname: wpool.tile([128, n_kd * DGl], MMDT, name=f"{wname}_sb")
            for wname in ("wq", "wk", "wv")
        }

        def load_w(wname, wap):
            nc.sync.dma_start(
                w_sbs[wname][:].rearrange("p (k f) -> p k f", k=n_kd),
                _r(wap).rearrange("(k p) f -> p k f", p=128),
            )

        def load_w_mblock(wname, wap, m):
            nc.sync.dma_start(
                w_sbs[wname][:].rearrange(
                    "p (k g j) -> p k g j", k=n_kd, j=128
                )[:, :, m, :],
                _r(wap).rearrange("(k p) (g j) -> p k g j", p=128, j=128)[
                    :, :, m, :
                ],
            )

        def load_xt(ns):
            # two k-half DMAs: the slice's first k-accumulations can start
            # as soon as the first half lands
            t = xpool.tile([128, n_kd * QB1], MMDT, tag="xt", name="xt_sb")
            half = n_kd // 2
            for hlf in range(2):
                nc.sync.dma_start(
                    t[:, ds(hlf * half * QB1, half * QB1)].rearrange(
                        "p (k f) -> p k f", k=half
                    ),
                    _r(
                        xt[ds(hlf * half * 128, half * 128), ts(ns, QB1)]
                    ).rearrange("(k p) f -> p k f", p=128),
                )
            return t

        nc.sync.dma_start(ones_sb[:], _r(aps["ones"]))
        nc.sync.dma_start(bq_sb[:], aps["bq"])
        nc.sync.dma_start(bk_sb[:], aps["bk"])
        nc.sync.dma_start(bv_sb[:], aps["bv"])
        # k^T first: PE can start on wk+x0 while wq/wv still stream in
        load_w_mblock("wk", wkt, 0)
        g0 = load_xt(0)
        for m in range(1, n_dg):
            load_w_mblock("wk", wkt, m)
        g1 = load_xt(1)
        for m in range(n_dg):
            load_w_mblock("wq", wqt, m)
        load_w("wv", wvt)
        nc.sync.dma_start(
            masks_sb[:].rearrange("p (i f) -> p i f", i=4),
            aps["masks"].rearrange("i p f -> p i f"),
        )

        def do_proj_t(res, wname, b_sb, ns, xt_sb):
            # q^T/k^T [m hd-dims 128, QB1 seq] drains into resident tiles
            for m in range(n_dg):
                ps = psum1.tile([128, QB1], F32, tag="ps", name="ps_qk")
                for k in range(n_kd):
                    nc.tensor.matmul(
                        ps[:],
                        lhsT=w_sbs[wname][:, ds(k * DGl + m * 128, 128)],
                        rhs=xt_sb[:, ts(k, QB1)],
                        start=(k == 0),
                        stop=(k == n_kd - 1),
                    )
                nc.scalar.activation(
                    res[m][:, ts(ns, QB1)],
                    ps[:],
                    ACTF.Identity,
                    bias=b_sb[:, ds(m, 1)],
                )

        def do_v(ns, xt_sb):
            for msub in range(QB1 // 128):
                ps = psum1.tile([128, DGl], F32, tag="ps", name="ps_v")
                for k in range(n_kd):
                    nc.tensor.matmul(
                        ps[:],
                        lhsT=xt_sb[:, ds(k * QB1 + msub * 128, 128)],
                        rhs=w_sbs["wv"][:, ts(k, DGl)],
                        start=(k == 0),
                        stop=(k == n_kd - 1),
                    )
                st = stage.tile([128, DGl], F32, tag="v_st", name="v_st")
                nc.vector.tensor_add(st[:], ps[:], bv_sb[:])
                nc.sync.dma_start(
                    v_d[ds(ns * QB1 + msub * 128, 128), :], st[:]
                )

        # head group: k^T for slices 0-1 (no DMA drains), then q^T, then v
        for ns, g in ((0, g0), (1, g1)):
            do_proj_t(kt_res, "wk", bk_sb, ns, g)
        for ns, g in ((0, g0), (1, g1)):
            do_proj_t(qt_res, "wq", bq_sb, ns, g)
        for ns, g in ((0, g0), (1, g1)):
            do_v(ns, g)
        nxt = load_xt(2) if n_ns > 2 else None
        for ns in range(2, n_ns):
            xt_sb = nxt
            nxt = load_xt(ns + 1) if ns + 1 < n_ns else None
            do_proj_t(kt_res, "wk", bk_sb, ns, xt_sb)
            do_proj_t(qt_res, "wq", bq_sb, ns, xt_sb)
            do_v(ns, xt_sb)

    # ---------------- Phase 2: causal attention ----------------
    # ctx^T per head stays resident in SBUF for phase 3
    ctx_pool = ctx.enter_context(tc.tile_pool(name="ctx_pool", bufs=1))
    ctx_sbs = [
        ctx_pool.tile([128, S], MMDT, tag=f"ctx{h}", name=f"ctx_sb{h}")
        for h in range(n_dg)
    ]

    # wo stays resident; loaded mid-phase-2 so phase 3 starts hot
    wopool = ctx.enter_context(tc.tile_pool(name="wo_pool", bufs=1))
    wo_sb = wopool.tile([128, n_dg * D], MMDT, name="wo_sb")

    # phase-2/3-only constants live after phase-1 pools are freed
    p2consts = ctx.enter_context(tc.tile_pool(name="p2consts", bufs=1))
    bo_sb = p2consts.tile([128, D], F32, name="bo_sb")
    nc.sync.dma_start(bo_sb[:], aps["bo"])

    with (
        tc.tile_pool(name="v_pool", bufs=3) as vpool,
        tc.tile_pool(name="exp_pool", bufs=8) as epool,
        tc.tile_pool(name="lrec_pool", bufs=3) as lpool,
        tc.tile_pool(name="bc_pool", bufs=3) as bcpool,
        tc.tile_pool(name="ps_s", bufs=5, space="PSUM") as ps_s_pool,
        tc.tile_pool(name="ps_c", bufs=2, space="PSUM") as ps_c_pool,
        tc.tile_pool(name="ps_l", bufs=1, space="PSUM") as ps_l_pool,
    ):
        for h in range(n_dg):
            # v_sb[p, t*128+j] = v[t*128+p, h*128+j]; quarter DMAs so the
            # first q-blocks' PV can start before the whole head lands
            v_sb = vpool.tile([128, n_sk * 128], MMDT, tag="v", name="v_sb")
            nq = max(1, S // 512)
            for vq in range(nq):
                nc.sync.dma_start(
                    v_sb[:, ds(vq * 512, 512)].rearrange(
                        "p (t j) -> p t j", j=128
                    ),
                    _r(v_d[ds(vq * 512, 512), ts(h, 128)]).rearrange(
                        "(t p) j -> p t j", p=128
                    ),
                )
            if h == 1:
                # wo_sb[p, k*D + f] = wot[k*128+p, f] (phase-3 prefetch)
                nc.sync.dma_start(
                    wo_sb[:].rearrange("p (k f) -> p k f", k=n_dg),
                    _r(wot).rearrange("(k p) f -> p k f", p=128),
                )
            for qb in range(n_sq):
                n_kt = (qb + 1) * (QB // KT)  # causal: only k-tiles <= q
                ps_c = ps_c_pool.tile([128, QB], F32, tag="c", name="ps_c")
                ps_l = ps_l_pool.tile([1, QB], F32, tag="l", name="ps_l")
                diag0 = n_kt - (QB // KT)
                for kt in range(n_kt):
                    off = kt - diag0
                    # causal column restriction: diagonal tile off needs
                    # only cols >= off*128; keep moving dim >= 256 for
                    # full-rate f32r (so off=3 starts at 256, masked).
                    sc = 0 if off < 1 else (128 if off == 1 else 256)
                    w = QB - sc
                    ps_sc = ps_s_pool.tile([128, QB], F32, tag="s", name="ps_sc")
                    nc.tensor.matmul(
                        ps_sc[:, ds(sc, w)],
                        lhsT=kt_res[h][:, ts(kt, 128)],
                        rhs=qt_res[h][:, ds(qb * QB + sc, w)],
                        start=True,
                        stop=True,
                    )
                    if off >= 0:
                        # only the triangular block (plus, for off=3, the
                        # fully-invalid 128 cols kept for moving-dim>=256)
                        # needs masking; columns right of it are all-valid
                        msc = off * 128 if off < 3 else 256
                        mw = 128 if off < 3 else 256
                        nc.vector.tensor_add(
                            ps_sc[:, ds(msc, mw)],
                            ps_sc[:, ds(msc, mw)],
                            masks_sb[:, ds(off * QB + msc, mw)],
                        )
                    ex = epool.tile([128, QB], MMDT, tag="e", name="ex")
                    nc.scalar.activation(
                        ex[:, ds(sc, w)], ps_sc[:, ds(sc, w)], ACTF.Exp, scale=SCALE
                    )
                    nc.tensor.matmul(
                        ps_c[:, ds(sc, w)],
                        lhsT=v_sb[:, ts(kt, 128)],
                        rhs=ex[:, ds(sc, w)],
                        start=(kt == 0),
                        stop=(kt == n_kt - 1),
                        skip_group_check=True,
                    )
                    nc.tensor.matmul(
                        ps_l[:, ds(sc, w)],
                        lhsT=ones_sb[:],
                        rhs=ex[:, ds(sc, w)],
                        start=(kt == 0),
                        stop=(kt == n_kt - 1),
                        skip_group_check=True,
                    )
                rec = lpool.tile([1, QB], F32, tag="r", name="rec")
                nc.vector.reciprocal(rec[:], ps_l[:])
                bc = bcpool.tile([128, QB], F32, tag="bc", name="bc")
                nc.gpsimd.partition_broadcast(bc[:], rec[:])
                nc.vector.tensor_mul(
                    ctx_sbs[h][:, ts(qb, QB)], ps_c[:], bc[:]
                )

    # ---------------- Phase 3: output projection ----------------
    with (
        tc.tile_pool(name="o_stage", bufs=4) as ostage,
        tc.tile_pool(name="p3_psum", bufs=4, space="PSUM") as psum3,
    ):
        for m in range(n_sk):
            for n in range(D // QB):
                ps = psum3.tile([128, QB], F32, tag="o", name="ps_p3")
                for k in range(n_dg):
                    nc.tensor.matmul(
                        ps[:],
                        lhsT=ctx_sbs[k][:, ts(m, 128)],
                        rhs=wo_sb[:, ds(k * D + n * QB, QB)],
                        start=(k == 0),
                        stop=(k == n_dg - 1),
                    )
                ot = ostage.tile([128, QB], F32, tag="ot", name="ot")
                nc.vector.tensor_add(ot[:], ps[:], bo_sb[:, ts(n, QB)])
                nc.sync.dma_start(out[ts(m, 128), ts(n, QB)], ot[:])


def build_program(S=SEQ, D=D_MODEL, DGl=DG, enable_asserts=False):
    nc = bacc.Bacc(
        "TRN2",
        target_bir_lowering=False,
        debug=False,
        enable_asserts=enable_asserts,
        num_devices=N_CORES,
    )
    aps = {
        "xt": nc.dram_tensor("xt", [D, S], F32, kind="ExternalInput").ap(),
        "wqt": nc.dram_tensor("wqt", [D, DGl], F32, kind="ExternalInput").ap(),
        "wkt": nc.dram_tensor("wkt", [D, DGl], F32, kind="ExternalInput").ap(),
        "wvt": nc.dram_tensor("wvt", [D, DGl], F32, kind="ExternalInput").ap(),
        "wot": nc.dram_tensor("wot", [DGl, D], F32, kind="ExternalInput").ap(),
        "bq": nc.dram_tensor("bq", [128, DGl // 128], F32, kind="ExternalInput").ap(),
        "bk": nc.dram_tensor("bk", [128, DGl // 128], F32, kind="ExternalInput").ap(),
        "bv": nc.dram_tensor("bv", [128, DGl], F32, kind="ExternalInput").ap(),
        "bo": nc.dram_tensor("bo", [128, D], F32, kind="ExternalInput").ap(),
        "masks": nc.dram_tensor("masks", [4, 128, QB], F32, kind="ExternalInput").ap(),
        "ones": nc.dram_tensor("ones", [128, 1], F32, kind="ExternalInput").ap(),
        "out": nc.dram_tensor("out", [S, D], F32, kind="ExternalOutput").ap(),
    }
    with tile.TileContext(nc) as tc:
        with ExitStack() as ctx:
            _mha_body(ctx, tc, aps, S, D, DGl)
    nc.compile()
    return nc


def make_masks():
    """Additive causal masks: 0 where k<=q, -1e30 where masked."""
    i = np.arange(4)[:, None, None]
    p = np.arange(128)[None, :, None]
    f = np.arange(QB)[None, None, :]
    keep = (i * 128 + p) <= f
    return np.where(keep, 0.0, -1e30).astype(np.float32)


def shard_inputs(x, wq, bq, wk, bk, wv, bv, wo, bo):
    """Build the 8 per-core input maps (host-side layout prep)."""
    masks = make_masks()
    xts = [np.ascontiguousarray(np.asarray(x[b], np.float32).T) for b in range(BATCH)]
    bo_bc = np.ascontiguousarray(
        np.broadcast_to(np.asarray(bo, np.float32), (128, D_MODEL))
    )
    bo_zero = np.zeros((128, D_MODEL), np.float32)
    in_maps = []
    for c in range(N_CORES):
        b, g = divmod(c, N_GROUPS)
        sl = slice(g * DG, (g + 1) * DG)
        in_maps.append(
            {
                "xt": xts[b],
                "wqt": np.ascontiguousarray(np.asarray(wq, np.float32)[sl].T),
                "wkt": np.ascontiguousarray(np.asarray(wk, np.float32)[sl].T),
                "wvt": np.ascontiguousarray(np.asarray(wv, np.float32)[sl].T),
                "wot": np.ascontiguousarray(np.asarray(wo, np.float32)[:, sl].T),
                "bq": np.ascontiguousarray(
                    np.asarray(bq, np.float32)[sl].reshape(-1, 128).T
                ),
                "bk": np.ascontiguousarray(
                    np.asarray(bk, np.float32)[sl].reshape(-1, 128).T
                ),
                "bv": np.ascontiguousarray(
                    np.broadcast_to(np.asarray(bv, np.float32)[sl], (128, DG))
                ),
                "bo": bo_bc if g == 0 else bo_zero,
                "masks": masks,
                "ones": np.ones((128, 1), np.float32),
            }
        )
    return in_maps


_NC_CACHE = {}


def get_program():
    if "nc" not in _NC_CACHE:
        _NC_CACHE["nc"] = build_program()
    return _NC_CACHE["nc"]


def run_sharded(inputs, trace=False):
    nc = get_program()
    in_maps = shard_inputs(**inputs)
    res = run_bass_kernel_spmd(nc, in_maps, list(range(N_CORES)), trace=trace)
    full = np.empty((BATCH, SEQ, D_MODEL), np.float32)
    for b in range(BATCH):
        acc = res.results[b * N_GROUPS]["out"].copy()
        for g in range(1, N_GROUPS):
            acc += res.results[b * N_GROUPS + g]["out"]
        full[b] = acc
    return full, res


def kernel(**inputs):
    out, _ = run_sharded(inputs, trace=False)
    return out



# revision 27
# speedup vs baseline: 1.1586x; 1.1586x over previous
"""Multi-head causal attention block on 8 Trainium2 NeuronCores.

Sharding: tensor-parallel over heads (4 groups of 4 heads) x data-parallel
over batch (2). Core c -> (batch b=c//4, head-group g=c%4). Each core
computes q/k/v projections for its head group, causal attention for its 4
heads, and a partial output projection; the host sums the 4 partials per
batch. All layout transposes are done host-side.

One software-pipelined instruction stream: projections run over slice
pairs; once a pair's k/v/q are resident, that q-block's attention (whose
exp work makes it Activation-bound) is woven tile-by-tile into the next
pair's projection matmuls so the PE never waits on ACT. Output-projection
chunks are the filler for the last q-block. All matmul operands are bf16
(full PE rate at any moving width, half the DMA of f32); softmax
denominators accumulate on DVE; biases apply on Pool; the PE runs only
real matmul work.

Self-contained: hardcodes shapes for the 2x2048x2048, 16-head problem.
"""

from contextlib import ExitStack

import ml_dtypes
import numpy as np

import concourse.bass as bass
import concourse.tile as tile
from concourse import bacc, mybir
from concourse.bass import ds, ts
from concourse.bass_utils import run_bass_kernel_spmd

F32 = mybir.dt.float32
BF16 = mybir.dt.bfloat16
ACTF = mybir.ActivationFunctionType
BFNP = ml_dtypes.bfloat16

# Full-problem dims
BATCH = 2
SEQ = 2048
D_MODEL = 2048
NUM_HEADS = 16
HEAD_DIM = 128
N_CORES = 8
N_GROUPS = 4  # head-groups (tensor parallel)
DG = D_MODEL // N_GROUPS  # 512 = 4 heads per group
SCALE = 1.0 / float(np.sqrt(HEAD_DIM))

QB1 = 512  # projection seq-slice width (512-row matmuls: SEQ-dispatch
#            per PE instruction is ~142ns, so 256-row/107ns matmuls are
#            sequencer-bound; 512-row/213ns are not)
N_SL = SEQ // QB1  # 4 slices == 4 q-blocks
NKD = D_MODEL // 128  # 16 contraction tiles over d_model
QB = 512  # attention q-block width
N_QB = SEQ // QB  # 4 q-blocks
N_DG = DG // 128  # 4 head tiles per group
N_SK = SEQ // 128  # 16 seq tiles


def _mha_body(ctx, tc, aps):
    nc = tc.nc
    S, D, DGl = SEQ, D_MODEL, DG
    xt, wqt, wkt, wvt, wot = (aps[k] for k in ("xt", "wqt", "wkt", "wvt", "wot"))
    out = aps["out"]

    consts = ctx.enter_context(tc.tile_pool(name="consts", bufs=1))
    ones_sb = consts.tile([128, 1], BF16, name="ones_sb")
    bq_sb = consts.tile([128, N_DG], F32, name="bq_sb")
    bk_sb = consts.tile([128, N_DG], F32, name="bk_sb")
    bv_sb = consts.tile([128, DGl], F32, name="bv_sb")
    tri_sb = consts.tile([128, 128], BF16, name="tri_sb")

    wpool = ctx.enter_context(tc.tile_pool(name="wpool", bufs=1))
    w_sbs = {
        w: wpool.tile([128, NKD * DGl], BF16, name=f"{w}_sb")
        for w in ("wq", "wk", "wv")
    }
    wo_sb = wpool.tile([128, N_DG * D], BF16, name="wo_sb")

    res = ctx.enter_context(tc.tile_pool(name="res", bufs=1))
    kt_res = [res.tile([128, S], BF16, tag=f"kt{m}", name=f"kt{m}") for m in range(N_DG)]
    qt_res = [res.tile([128, S], BF16, tag=f"qt{m}", name=f"qt{m}") for m in range(N_DG)]
    v_all = res.tile([128, N_SK * DGl], BF16, name="v_all")  # [p, t*DGl + j]
    ctx_sbs = [res.tile([128, S], BF16, tag=f"cx{m}", name=f"cx{m}") for m in range(N_DG)]

    xpool = ctx.enter_context(tc.tile_pool(name="xpool", bufs=2))
    epool = ctx.enter_context(tc.tile_pool(name="epool", bufs=6))
    accp = ctx.enter_context(tc.tile_pool(name="accp", bufs=2))
    lrec = ctx.enter_context(tc.tile_pool(name="lrec", bufs=2))
    bcp = ctx.enter_context(tc.tile_pool(name="bcp", bufs=2))
    ost = ctx.enter_context(tc.tile_pool(name="ost", bufs=8))

    ps_s = ctx.enter_context(tc.tile_pool(name="ps_s", bufs=2, space="PSUM"))
    ps_c = ctx.enter_context(tc.tile_pool(name="ps_c", bufs=2, space="PSUM"))
    ps_l = ctx.enter_context(tc.tile_pool(name="ps_l", bufs=1, space="PSUM"))

    # ---------------- DMA issue helpers ----------------
    def load_w_part(wname, wap, k0, nk):
        nc.sync.dma_start(
            w_sbs[wname][:, ds(k0 * DGl, nk * DGl)].rearrange("p (k f) -> p k f", k=nk),
            wap.rearrange("(k p) f -> p k f", p=128)[:, ds(k0, nk), :],
        )

    def load_wo(k0, nk):
        nc.sync.dma_start(
            wo_sb[:, ds(k0 * D, nk * D)].rearrange("p (k f) -> p k f", k=nk),
            wot.rearrange("(k p) f -> p k f", p=128)[:, ds(k0, nk), :],
        )

    def load_x(s, k_chunks=(8, 8)):
        t = xpool.tile([128, NKD * QB1], BF16, tag="xt", name="xt_sb")
        k0 = 0
        for nk in k_chunks:
            nc.sync.dma_start(
                t[:, ds(k0 * QB1, nk * QB1)].rearrange("p (k f) -> p k f", k=nk),
                xt[ds(k0 * 128, nk * 128), ts(s, QB1)].rearrange(
                    "(k p) f -> p k f", p=128
                ),
            )
            k0 += nk
        return t

    # ---------------- instruction-stream generators ----------------
    # Each generator emits instructions as it is advanced; one `yield` per
    # matmul (or drain) so the weaver can interleave streams finely.

    def gen_kq(dst, wname, b_sb, s, x_sb, pj):
        # q^T/k^T [head-tile m: 128 hd-dims, QB1 seq] -> resident tiles
        for m in range(N_DG):
            ps = pj.tile([128, QB1], F32, tag="pj", name="ps_kq")
            for k in range(NKD):
                nc.tensor.matmul(
                    ps[:],
                    lhsT=w_sbs[wname][:, ds(k * DGl + m * 128, 128)],
                    rhs=x_sb[:, ts(k, QB1)],
                    start=(k == 0),
                    stop=(k == NKD - 1),
                    skip_group_check=True,
                )
                yield
            nc.scalar.activation(
                dst[m][:, ts(s, QB1)], ps[:], ACTF.Identity,
                bias=b_sb[:, ds(m, 1)],
            )
            yield

    def gen_v(s, x_sb, pj):
        # v [seq 128, DGl] -> v_all resident
        for msub in range(QB1 // 128):
            ps = pj.tile([128, DGl], F32, tag="pj", name="ps_v")
            for k in range(NKD):
                nc.tensor.matmul(
                    ps[:],
                    lhsT=x_sb[:, ds(k * QB1 + msub * 128, 128)],
                    rhs=w_sbs["wv"][:, ts(k, DGl)],
                    start=(k == 0),
                    stop=(k == NKD - 1),
                    skip_group_check=True,
                )
                yield
            t = (QB1 // 128) * s + msub
            nc.vector.tensor_add(v_all[:, ds(t * DGl, DGl)], ps[:], bv_sb[:])
            yield

    def gen_slice(s, x_sb, pj):
        yield from gen_kq(kt_res, "wk", bk_sb, s, x_sb, pj)
        yield from gen_v(s, x_sb, pj)
        yield from gen_kq(qt_res, "wq", bq_sb, s, x_sb, pj)

    SLICE_STEPS = 3 * 4 * (NKD + 1)  # 204

    def gen_p3(qb, pj3):
        # output projection for the 4 seq tiles of q-block qb. bo is added
        # host-side, so q-blocks 0-2 (woven into round 4) DMA the psum
        # straight to DRAM; q-block 3 (round 5, back-to-back groups)
        # stages through an ACT copy so the psum bank frees in ~0.7us
        # instead of being held for the ~3us DMA round-trip.
        for mi in range(4):
            m = qb * 4 + mi
            for n in range(D // QB):
                ps = pj3.tile([128, QB], F32, tag="p3", name="ps_p3")
                for g in range(N_DG):
                    nc.tensor.matmul(
                        ps[:],
                        lhsT=ctx_sbs[g][:, ts(m, 128)],
                        rhs=wo_sb[:, ds(g * D + n * QB, QB)],
                        start=(g == 0),
                        stop=(g == N_DG - 1),
                        skip_group_check=True,
                    )
                    yield
                last = qb == 3 and mi == 3 and n == D // QB - 1
                if qb == 3:
                    ot = ost.tile([128, QB], F32, tag="ot", name="ot")
                    if last:
                        for hlf in range(2):
                            nc.scalar.copy(
                                ot[:, ds(hlf * 256, 256)],
                                ps[:, ds(hlf * 256, 256)],
                            )
                            nc.sync.dma_start(
                                out[ts(m, 128), ds(n * QB + hlf * 256, 256)],
                                ot[:, ds(hlf * 256, 256)],
                            )
                    else:
                        nc.scalar.copy(ot[:], ps[:])
                        nc.sync.dma_start(out[ts(m, 128), ts(n, QB)], ot[:])
                else:
                    # DMA cannot read PSUM; stage through a copy, split
                    # between ACT and DVE so neither engine saturates in
                    # round 4 (ACT also runs qb3's exp, DVE its softmax).
                    ot = ost.tile([128, QB], F32, tag="ot", name="ot")
                    if (mi + n) % 2 == 0:
                        nc.scalar.copy(ot[:], ps[:])
                    else:
                        nc.vector.tensor_copy(ot[:], ps[:])
                    nc.sync.dma_start(out[ts(m, 128), ts(n, QB)], ot[:])
                yield

    P3_STEPS = 4 * (D // QB) * (N_DG + 1)  # 80 per q-block

    def attn_unit(h, qb):
        # causal attention for head-tile h over q-block qb; softmax
        # denominator accumulates on DVE (no PE ones-matmul per k-tile);
        # PV for tile t is emitted with scores of tile t+1 so the exp
        # latency is covered by interleaved filler matmuls.
        n_kt = 4 * (qb + 1)
        diag0 = n_kt - 4
        pc = ps_c.tile([128, QB], F32, tag="c", name="ps_c")
        acc = accp.tile([128, QB], BF16, tag="acc", name="acc")

        def emit_pv(kt, sc, w, ex):
            nc.tensor.matmul(
                pc[:, ds(sc, w)],
                lhsT=v_all[:, ds(kt * DGl + h * 128, 128)],
                rhs=ex[:, ds(sc, w)],
                start=(kt == 0),
                stop=(kt == n_kt - 1),
                skip_group_check=True,
            )
            if kt == 0:
                nc.vector.tensor_copy(acc[:], ex[:])
            else:
                nc.vector.tensor_add(
                    acc[:, ds(sc, w)], acc[:, ds(sc, w)], ex[:, ds(sc, w)]
                )

        # software pipeline depth 2: PV for tile t issues two stages after
        # its scores, so the scores->mask->exp chain (~1.3us) is covered
        # even when only ~3 filler matmuls separate stages (round 4).
        pend = []
        for kt in range(n_kt):
            off = kt - diag0
            sc = max(0, off) * 128
            w = QB - sc
            pss = ps_s.tile([128, QB], F32, tag="s", name="ps_s")
            nc.tensor.matmul(
                pss[:, ds(sc, w)],
                lhsT=kt_res[h][:, ts(kt, 128)],
                rhs=qt_res[h][:, ds(qb * QB + sc, w)],
                start=True,
                stop=True,
            )
            ex = epool.tile([128, QB], BF16, tag="ex", name="ex")
            nc.scalar.activation(ex[:, ds(sc, w)], pss[:, ds(sc, w)], ACTF.Exp, scale=SCALE)
            if off >= 0:
                # zero the masked upper triangle after exp: keeps the
                # scores->exp chain free of any DVE hop (ps_s turnaround
                # gates the 2-deep scores pipeline); PV reads ex two
                # stages later, so this mul is far off the critical path.
                nc.vector.tensor_mul(
                    ex[:, ds(sc, 128)], ex[:, ds(sc, 128)], tri_sb[:]
                )
            pend.append((kt, sc, w, ex))
            if len(pend) > 2:
                emit_pv(*pend.pop(0))
            yield
        for p_ in pend:
            emit_pv(*p_)
        pl = ps_l.tile([1, QB], F32, tag="l", name="ps_l")
        nc.tensor.matmul(pl[:], lhsT=ones_sb[:], rhs=acc[:], start=True, stop=True)
        rec = lrec.tile([1, QB], F32, tag="r", name="rec")
        nc.vector.reciprocal(rec[:], pl[:])
        bc = bcp.tile([128, QB], F32, tag="bc", name="bc")
        nc.gpsimd.partition_broadcast(bc[:], rec[:])
        nc.vector.tensor_mul(ctx_sbs[h][:, ts(qb, QB)], pc[:], bc[:])
        yield

    ATTN_STEPS = lambda qb: 4 * (qb + 1) + 1

    def weave(units, n_unit_steps, filler, n_filler_steps):
        # Bresenham-distribute filler steps across attention unit stages.
        # A burst of fillers at each unit boundary covers the previous
        # unit's still-in-flight exp tiles (ps_s buffer reuse) so the new
        # unit's first scores matmul doesn't stall the PE.
        err = 0
        for u in units:
            for b in range(5):
                if next(filler, None) is None:
                    break
                err -= n_unit_steps
            for _ in u:
                err += n_filler_steps
                while err >= n_unit_steps:
                    err -= n_unit_steps
                    if next(filler, None) is None:
                        err = -(1 << 30)
        for _ in filler:
            pass

    # ---------------- top-level schedule ----------------
    # The DMA device is serial in the sim (~1.46us per 4KB/line chunk), so
    # arrival order must track PE consumption order: a sliver of wk to
    # start, bias consts early (psum drains need them!), x slice 0, the
    # rest of wk, then wv / wq / x slice 1. A warmup matmul block finishes
    # the PE clock ramp while the first loads are in flight.
    load_w_part("wk", wkt, 0, 4)
    load_w_part("wk", wkt, 4, 6)
    load_w_part("wk", wkt, 10, 6)
    xa = load_x(0, k_chunks=(8, 8))
    nc.sync.dma_start(bk_sb[:], aps["bk"])
    nc.sync.dma_start(bq_sb[:], aps["bq"])
    nc.sync.dma_start(bv_sb[:], aps["bv"])
    nc.sync.dma_start(tri_sb[:], aps["tri"])
    nc.sync.dma_start(ones_sb[:], aps["ones"])
    load_w_part("wv", wvt, 0, 8)
    load_w_part("wv", wvt, 8, 8)
    load_w_part("wq", wqt, 0, 8)
    load_w_part("wq", wqt, 8, 8)
    xb = load_x(1)

    # warmup: the serial DMA device needs ~13us to deliver wk + x slice 0,
    # so run dummy matmuls until then. This both hides the DMA preamble
    # and finishes the PE clock ramp (3us of continuous use) so the real
    # stream starts at full speed with no gap (any PE idle gap resets the
    # ramp and costs ~1us of mid-p-state time).
    with tc.tile_pool(name="warm", bufs=1) as wrm, tc.tile_pool(
        name="warm_ps", bufs=1, space="PSUM"
    ) as wps:
        wtile = wrm.tile([128, 512], BF16, name="warm_sb")
        nc.gpsimd.memset(wtile[:], 0.0)
        wp = wps.tile([128, 512], F32, tag="w", name="warm_ps")
        NWARM = 34
        for i in range(NWARM):
            nc.tensor.matmul(
                wp[:],
                lhsT=wtile[:, ds(0, 128)],
                rhs=wtile[:],
                start=(i == 0),
                stop=(i == NWARM - 1),
            )

    with tc.tile_pool(name="pj", bufs=2, space="PSUM") as pj:
        # round 0: projections for slice 0, no attention yet
        for _ in gen_slice(0, xa, pj):
            pass
        # rounds 1-3: slice P woven with attention over q-block P-1
        for P in (1, 2, 3):
            xa = xb
            xb = load_x(P + 1) if P < 3 else None
            if P == 2:
                load_wo(0, 2)
                load_wo(2, 2)
            qb = P - 1
            units = [attn_unit(h, qb) for h in range(N_DG)]
            weave(units, N_DG * ATTN_STEPS(qb), gen_slice(P, xa, pj), SLICE_STEPS)

    # round 4: last q-block's attention woven with output projection of
    # q-blocks 0-2; then the remaining output projection.
    with tc.tile_pool(name="pj3", bufs=3, space="PSUM") as pj3:
        units = [attn_unit(h, 3) for h in range(N_DG)]

        def p3_fill():
            for qb in range(3):
                yield from gen_p3(qb, pj3)

        weave(units, N_DG * ATTN_STEPS(3), p3_fill(), 3 * P3_STEPS)
        for _ in gen_p3(3, pj3):
            pass


def build_program(enable_asserts=False):
    nc = bacc.Bacc(
        "TRN2",
        target_bir_lowering=False,
        debug=False,
        enable_asserts=enable_asserts,
        num_devices=N_CORES,
    )
    S, D, DGl = SEQ, D_MODEL, DG
    aps = {
        "xt": nc.dram_tensor("xt", [D, S], BF16, kind="ExternalInput").ap(),
        "wqt": nc.dram_tensor("wqt", [D, DGl], BF16, kind="ExternalInput").ap(),
        "wkt": nc.dram_tensor("wkt", [D, DGl], BF16, kind="ExternalInput").ap(),
        "wvt": nc.dram_tensor("wvt", [D, DGl], BF16, kind="ExternalInput").ap(),
        "wot": nc.dram_tensor("wot", [DGl, D], BF16, kind="ExternalInput").ap(),
        "bq": nc.dram_tensor("bq", [128, DGl // 128], F32, kind="ExternalInput").ap(),
        "bk": nc.dram_tensor("bk", [128, DGl // 128], F32, kind="ExternalInput").ap(),
        "bv": nc.dram_tensor("bv", [128, DGl], F32, kind="ExternalInput").ap(),
        "tri": nc.dram_tensor("tri", [128, 128], BF16, kind="ExternalInput").ap(),
        "ones": nc.dram_tensor("ones", [128, 1], BF16, kind="ExternalInput").ap(),
        "out": nc.dram_tensor("out", [S, D], F32, kind="ExternalOutput").ap(),
    }
    with tile.TileContext(nc) as tc:
        with ExitStack() as ctx:
            _mha_body(ctx, tc, aps)
    nc.compile()
    return nc


def make_tri():
    """Multiplicative causal mask for the 128x128 diagonal block: 1 where
    kpos<=qpos (keep), 0 where masked (applied to exp'd scores)."""
    p = np.arange(128)[:, None]
    f = np.arange(128)[None, :]
    return np.where(p <= f, 1.0, 0.0).astype(BFNP)


def shard_inputs(x, wq, bq, wk, bk, wv, bv, wo, bo):
    """Build the 8 per-core input maps (host-side layout prep + bf16)."""
    tri = make_tri()
    xts = [
        np.ascontiguousarray(np.asarray(x[b], np.float32).T).astype(BFNP)
        for b in range(BATCH)
    ]
    in_maps = []
    for c in range(N_CORES):
        b, g = divmod(c, N_GROUPS)
        sl = slice(g * DG, (g + 1) * DG)
        in_maps.append(
            {
                "xt": xts[b],
                "wqt": np.ascontiguousarray(np.asarray(wq, np.float32)[sl].T).astype(BFNP),
                "wkt": np.ascontiguousarray(np.asarray(wk, np.float32)[sl].T).astype(BFNP),
                "wvt": np.ascontiguousarray(np.asarray(wv, np.float32)[sl].T).astype(BFNP),
                "wot": np.ascontiguousarray(np.asarray(wo, np.float32)[:, sl].T).astype(BFNP),
                "bq": np.ascontiguousarray(
                    np.asarray(bq, np.float32)[sl].reshape(-1, 128).T
                ),
                "bk": np.ascontiguousarray(
                    np.asarray(bk, np.float32)[sl].reshape(-1, 128).T
                ),
                "bv": np.ascontiguousarray(
                    np.broadcast_to(np.asarray(bv, np.float32)[sl], (128, DG))
                ),
                "tri": tri,
                "ones": np.ones((128, 1), BFNP),
            }
        )
    return in_maps


_NC_CACHE = {}


def get_program():
    if "nc" not in _NC_CACHE:
        _NC_CACHE["nc"] = build_program()
    return _NC_CACHE["nc"]


def run_sharded(inputs, trace=False):
    nc = get_program()
    in_maps = shard_inputs(**inputs)
    res = run_bass_kernel_spmd(nc, in_maps, list(range(N_CORES)), trace=trace)
    bo = np.asarray(inputs["bo"], np.float32)
    full = np.empty((BATCH, SEQ, D_MODEL), np.float32)
    for b in range(BATCH):
        acc = res.results[b * N_GROUPS]["out"].copy()
        for g in range(1, N_GROUPS):
            acc += res.results[b * N_GROUPS + g]["out"]
        full[b] = acc + bo
    return full, res


def kernel(**inputs):
    out, _ = run_sharded(inputs, trace=False)
    return out


# revision 36
# speedup vs baseline: 1.1894x; 1.0265x over previous
"""Multi-head causal attention block on 8 Trainium2 NeuronCores.

Sharding: tensor-parallel over heads (4 groups of 4 heads) x data-parallel
over batch (2). Core c -> (batch b=c//4, head-group g=c%4). Each core
computes q/k/v projections for its head group, causal attention for its 4
heads, and a partial output projection; the host sums the 4 partials per
batch. All layout transposes are done host-side.

One software-pipelined instruction stream: projections run over slice
pairs; once a pair's k/v/q are resident, that q-block's attention (whose
exp work makes it Activation-bound) is woven tile-by-tile into the next
pair's projection matmuls so the PE never waits on ACT. Output-projection
chunks are the filler for the last q-block. All matmul operands are bf16
(full PE rate at any moving width, half the DMA of f32); softmax
denominators accumulate on DVE; biases apply on Pool; the PE runs only
real matmul work.

Self-contained: hardcodes shapes for the 2x2048x2048, 16-head problem.
"""

from contextlib import ExitStack

import ml_dtypes
import numpy as np

import concourse.bass as bass
import concourse.tile as tile
from concourse import bacc, mybir
from concourse.bass import ds, ts
from concourse.bass_utils import run_bass_kernel_spmd

F32 = mybir.dt.float32
BF16 = mybir.dt.bfloat16
FP8 = mybir.dt.float8e4
ACTF = mybir.ActivationFunctionType
BFNP = ml_dtypes.bfloat16

# Full-problem dims
BATCH = 2
SEQ = 2048
D_MODEL = 2048
NUM_HEADS = 16
HEAD_DIM = 128
N_CORES = 8
N_GROUPS = 4  # head-groups (tensor parallel)
DG = D_MODEL // N_GROUPS  # 512 = 4 heads per group
SCALE = 1.0 / float(np.sqrt(HEAD_DIM))

QB1 = 512  # projection seq-slice width (512-row matmuls: SEQ-dispatch
#            per PE instruction is ~142ns, so 256-row/107ns matmuls are
#            sequencer-bound; 512-row/213ns are not)
N_SL = SEQ // QB1  # 4 slices == 4 q-blocks
NKD = D_MODEL // 128  # 16 contraction tiles over d_model
QB = 512  # attention q-block width
N_QB = SEQ // QB  # 4 q-blocks
N_DG = DG // 128  # 4 head tiles per group
N_SK = SEQ // 128  # 16 seq tiles


def _mha_body(ctx, tc, aps):
    nc = tc.nc
    S, D, DGl = SEQ, D_MODEL, DG
    xt, wqt, wkt, wvt, wot = (aps[k] for k in ("xt", "wqt", "wkt", "wvt", "wot"))
    out = aps["out"]

    consts = ctx.enter_context(tc.tile_pool(name="consts", bufs=1))
    ones_sb = consts.tile([128, 1], BF16, name="ones_sb")
    bq_sb = consts.tile([128, N_DG], F32, name="bq_sb")
    bk_sb = consts.tile([128, N_DG], F32, name="bk_sb")
    bv_sb = consts.tile([128, DGl], F32, name="bv_sb")
    tri_sb = consts.tile([128, 128], BF16, name="tri_sb")

    wpool = ctx.enter_context(tc.tile_pool(name="wpool", bufs=1))
    w_sbs = {
        w: wpool.tile([128, NKD * DGl], BF16, name=f"{w}_sb")
        for w in ("wq", "wk", "wv")
    }
    wo_sb = wpool.tile([128, N_DG * D], BF16, name="wo_sb")

    res = ctx.enter_context(tc.tile_pool(name="res", bufs=1))
    # q^T/k^T feed only the scores matmul, which runs fp8e4m3 in DoubleRow
    # mode (0.5 PE cycles/row): packed layout [64 partitions, 2, seq],
    # head-dim hd -> (j=hd//64 plane, p=hd%64). Projections drain to an
    # fp8 staging tile; an SBUF->SBUF DMA repacks partitions 64-127 into
    # plane 1 (engines can't move data across partitions, DMA can).
    kt_res = [res.tile([64, 2 * S], FP8, tag=f"kt{m}", name=f"kt{m}") for m in range(N_DG)]
    qt_res = [res.tile([64, 2 * S], FP8, tag=f"qt{m}", name=f"qt{m}") for m in range(N_DG)]
    v_all = res.tile([128, N_SK * DGl], BF16, name="v_all")  # [p, t*DGl + j]
    ctx_sbs = [res.tile([128, S], BF16, tag=f"cx{m}", name=f"cx{m}") for m in range(N_DG)]

    xpool = ctx.enter_context(tc.tile_pool(name="xpool", bufs=2))
    epool = ctx.enter_context(tc.tile_pool(name="epool", bufs=6))
    accp = ctx.enter_context(tc.tile_pool(name="accp", bufs=2))
    lrec = ctx.enter_context(tc.tile_pool(name="lrec", bufs=2))
    bcp = ctx.enter_context(tc.tile_pool(name="bcp", bufs=2))
    ost = ctx.enter_context(tc.tile_pool(name="ost", bufs=8))
    st8 = ctx.enter_context(tc.tile_pool(name="st8", bufs=4))


    # ---------------- DMA issue helpers ----------------
    def load_w_part(wname, wap, k0, nk):
        nc.sync.dma_start(
            w_sbs[wname][:, ds(k0 * DGl, nk * DGl)].rearrange("p (k f) -> p k f", k=nk),
            wap.rearrange("(k p) f -> p k f", p=128)[:, ds(k0, nk), :],
        )

    def load_wo(k0, nk):
        nc.sync.dma_start(
            wo_sb[:, ds(k0 * D, nk * D)].rearrange("p (k f) -> p k f", k=nk),
            wot.rearrange("(k p) f -> p k f", p=128)[:, ds(k0, nk), :],
        )

    def load_x(s, k_chunks=(8, 8)):
        t = xpool.tile([128, NKD * QB1], BF16, tag="xt", name="xt_sb")
        k0 = 0
        for nk in k_chunks:
            nc.sync.dma_start(
                t[:, ds(k0 * QB1, nk * QB1)].rearrange("p (k f) -> p k f", k=nk),
                xt[ds(k0 * 128, nk * 128), ts(s, QB1)].rearrange(
                    "(k p) f -> p k f", p=128
                ),
            )
            k0 += nk
        return t

    # ---------------- instruction-stream generators ----------------
    # Each generator emits instructions as it is advanced; one `yield` per
    # matmul (or drain) so the weaver can interleave streams finely.


    def drain_pack_qk(dst, ps_ap, b_ap, m, s):
        st = st8.tile([128, QB1], FP8, tag="st8", name="st8")
        nc.scalar.activation(st[:], ps_ap, ACTF.Identity, bias=b_ap)
        # one DMA per plane: a single AP whose partition index strides
        # across the 64/128 boundary reads garbage on hw, so plane j pulls
        # partitions [j*64, j*64+64) with a plain partition-base offset
        for j in range(2):
            nc.sync.dma_start(
                dst[m][:, ds(j * S + s * QB1, QB1)],
                st[ds(j * 64, 64), :],
            )

    def gen_kq(dst, wname, b_sb, s, x_sb, pj):
        # q^T/k^T [head-tile m: 128 hd-dims, QB1 seq] -> resident tiles
        for m in range(N_DG):
            ps = pj.tile([128, QB1], F32, tag="pj", name="ps_kq")
            for k in range(NKD):
                nc.tensor.matmul(
                    ps[:],
                    lhsT=w_sbs[wname][:, ds(k * DGl + m * 128, 128)],
                    rhs=x_sb[:, ts(k, QB1)],
                    start=(k == 0),
                    stop=(k == NKD - 1),
                    skip_group_check=True,
                )
                yield
            drain_pack_qk(dst, ps[:], b_sb[:, ds(m, 1)], m, s)
            yield

    def gen_v(s, x_sb, pj, tag="pj"):
        # v [seq 128, DGl] -> v_all resident
        for msub in range(QB1 // 128):
            ps = pj.tile([128, DGl], F32, tag=tag, name="ps_v")
            for k in range(NKD):
                nc.tensor.matmul(
                    ps[:],
                    lhsT=x_sb[:, ds(k * QB1 + msub * 128, 128)],
                    rhs=w_sbs["wv"][:, ts(k, DGl)],
                    start=(k == 0),
                    stop=(k == NKD - 1),
                    skip_group_check=True,
                )
                yield
            t = (QB1 // 128) * s + msub
            nc.vector.tensor_add(v_all[:, ds(t * DGl, DGl)], ps[:], bv_sb[:])
            yield

    def gen_slice(s, x_sb, pj):
        yield from gen_kq(kt_res, "wk", bk_sb, s, x_sb, pj)
        yield from gen_v(s, x_sb, pj)
        yield from gen_kq(qt_res, "wq", bq_sb, s, x_sb, pj)

    SLICE_STEPS = 3 * 4 * (NKD + 1)  # 204

    def gen_p3(qb, pj3):
        # output projection for the 4 seq tiles of q-block qb. bo is added
        # host-side, so q-blocks 0-2 (woven into round 4) DMA the psum
        # straight to DRAM; q-block 3 (round 5, back-to-back groups)
        # stages through an ACT copy so the psum bank frees in ~0.7us
        # instead of being held for the ~3us DMA round-trip.
        for mi in range(4):
            m = qb * 4 + mi
            for n in range(D // QB):
                ps = pj3.tile([128, QB], F32, tag="p3", name="ps_p3")
                for g in range(N_DG):
                    nc.tensor.matmul(
                        ps[:],
                        lhsT=ctx_sbs[g][:, ts(m, 128)],
                        rhs=wo_sb[:, ds(g * D + n * QB, QB)],
                        start=(g == 0),
                        stop=(g == N_DG - 1),
                        skip_group_check=True,
                    )
                    yield
                last = qb == 3 and mi == 3 and n == D // QB - 1
                if qb == 3:
                    ot = ost.tile([128, QB], F32, tag="ot", name="ot")
                    if last:
                        for hlf in range(2):
                            nc.scalar.copy(
                                ot[:, ds(hlf * 256, 256)],
                                ps[:, ds(hlf * 256, 256)],
                            )
                            nc.sync.dma_start(
                                out[ts(m, 128), ds(n * QB + hlf * 256, 256)],
                                ot[:, ds(hlf * 256, 256)],
                            )
                    else:
                        nc.scalar.copy(ot[:], ps[:])
                        nc.sync.dma_start(out[ts(m, 128), ts(n, QB)], ot[:])
                else:
                    # DMA cannot read PSUM; stage through a copy, split
                    # between ACT and DVE so neither engine saturates in
                    # round 4 (ACT also runs qb3's exp, DVE its softmax).
                    ot = ost.tile([128, QB], F32, tag="ot", name="ot")
                    if (mi + n) % 2 == 0:
                        nc.scalar.copy(ot[:], ps[:])
                    else:
                        nc.vector.tensor_copy(ot[:], ps[:])
                    nc.sync.dma_start(out[ts(m, 128), ts(n, QB)], ot[:])
                yield

    P3_STEPS = 4 * (D // QB) * (N_DG + 1)  # 80 per q-block

    def attn_unit(h, qb):
        # causal attention for head-tile h over q-block qb; softmax
        # denominator accumulates on DVE (no PE ones-matmul per k-tile);
        # PV for tile t is emitted with scores of tile t+1 so the exp
        # latency is covered by interleaved filler matmuls.
        n_kt = 4 * (qb + 1)
        diag0 = n_kt - 4
        pc = ps_c.tile([128, QB], F32, tag="c", name="ps_c")
        acc = accp.tile([128, QB], BF16, tag="acc", name="acc")

        def emit_pv(kt, sc, w, ex):
            nc.tensor.matmul(
                pc[:, ds(sc, w)],
                lhsT=v_all[:, ds(kt * DGl + h * 128, 128)],
                rhs=ex[:, ds(sc, w)],
                start=(kt == 0),
                stop=(kt == n_kt - 1),
                skip_group_check=True,
            )
            if kt == 0:
                nc.vector.tensor_copy(acc[:], ex[:])
            else:
                nc.vector.tensor_add(
                    acc[:, ds(sc, w)], acc[:, ds(sc, w)], ex[:, ds(sc, w)]
                )

        # software pipeline depth 2: PV for tile t issues two stages after
        # its scores, so the scores->mask->exp chain (~1.3us) is covered
        # even when only ~3 filler matmuls separate stages (round 4).
        pend = []
        for kt in range(n_kt):
            off = kt - diag0
            sc = max(0, off) * 128
            w = QB - sc
            pss = ps_s.tile([128, QB], F32, tag="s", name="ps_s")
            nc.tensor.matmul(
                pss[:, ds(sc, w)],
                lhsT=kt_res[h][:].rearrange("p (j s) -> p j s", j=2)[
                    :, :, ts(kt, 128)
                ],
                rhs=qt_res[h][:].rearrange("p (j s) -> p j s", j=2)[
                    :, :, ds(qb * QB + sc, w)
                ],
                start=True,
                stop=True,
                perf_mode=mybir.MatmulPerfMode.DoubleRow,
            )
            ex = epool.tile([128, QB], BF16, tag="ex", name="ex")
            nc.scalar.activation(ex[:, ds(sc, w)], pss[:, ds(sc, w)], ACTF.Exp, scale=SCALE)
            if off >= 0:
                # zero the masked upper triangle after exp: keeps the
                # scores->exp chain free of any DVE hop (ps_s turnaround
                # gates the 2-deep scores pipeline); PV reads ex two
                # stages later, so this mul is far off the critical path.
                nc.vector.tensor_mul(
                    ex[:, ds(sc, 128)], ex[:, ds(sc, 128)], tri_sb[:]
                )
            pend.append((kt, sc, w, ex))
            if len(pend) > 2:
                emit_pv(*pend.pop(0))
            yield
        for p_ in pend:
            emit_pv(*p_)
        pl = ps_l.tile([1, QB], F32, tag="l", name="ps_l")
        nc.tensor.matmul(pl[:], lhsT=ones_sb[:], rhs=acc[:], start=True, stop=True)
        rec = lrec.tile([1, QB], F32, tag="r", name="rec")
        nc.vector.reciprocal(rec[:], pl[:])
        bc = bcp.tile([128, QB], F32, tag="bc", name="bc")
        nc.gpsimd.partition_broadcast(bc[:], rec[:])
        nc.vector.tensor_mul(ctx_sbs[h][:, ts(qb, QB)], pc[:], bc[:])
        yield

    ATTN_STEPS = lambda qb: 4 * (qb + 1) + 1

    def weave(units, n_unit_steps, filler, n_filler_steps):
        # Bresenham-distribute filler steps across attention unit stages.
        # A burst of fillers at each unit boundary covers the previous
        # unit's still-in-flight exp tiles (ps_s buffer reuse) so the new
        # unit's first scores matmul doesn't stall the PE.
        err = 0
        for u in units:
            for b in range(5):
                if next(filler, None) is None:
                    break
                err -= n_unit_steps
            for _ in u:
                err += n_filler_steps
                while err >= n_unit_steps:
                    err -= n_unit_steps
                    if next(filler, None) is None:
                        err = -(1 << 30)
        for _ in filler:
            pass

    # ---------------- top-level schedule ----------------
    # The DMA device is serial in the sim (~1.46us per 4KB/line chunk), so
    # arrival order must track PE consumption order: a sliver of wk to
    # start, bias consts early (psum drains need them!), x slice 0, the
    # rest of wk, then wv / wq / x slice 1. A warmup matmul block finishes
    # the PE clock ramp while the first loads are in flight.
    load_w_part("wk", wkt, 0, 4)
    load_w_part("wk", wkt, 4, 6)
    load_w_part("wk", wkt, 10, 6)
    xa = load_x(0, k_chunks=(8, 8))
    nc.sync.dma_start(bk_sb[:], aps["bk"])
    nc.sync.dma_start(bq_sb[:], aps["bq"])
    nc.sync.dma_start(bv_sb[:], aps["bv"])
    nc.sync.dma_start(tri_sb[:], aps["tri"])
    nc.sync.dma_start(ones_sb[:], aps["ones"])
    load_w_part("wv", wvt, 0, 8)
    load_w_part("wv", wvt, 8, 8)
    load_w_part("wq", wqt, 0, 8)
    load_w_part("wq", wqt, 8, 8)
    xb = load_x(1)

    # warmup: the serial DMA device needs ~13us to deliver wk + x slice 0,
    # so run dummy matmuls until then. This both hides the DMA preamble
    # and finishes the PE clock ramp (3us of continuous use) so the real
    # stream starts at full speed with no gap (any PE idle gap resets the
    # ramp and costs ~1us of mid-p-state time).
    with tc.tile_pool(name="warm", bufs=1) as wrm, tc.tile_pool(
        name="warm_ps", bufs=1, space="PSUM"
    ) as wps:
        wtile = wrm.tile([128, 512], BF16, name="warm_sb")
        nc.gpsimd.memset(wtile[:], 0.0)
        wp = wps.tile([128, 512], F32, tag="w", name="warm_ps")
        NWARM = 24
        for i in range(NWARM):
            nc.tensor.matmul(
                wp[:],
                lhsT=wtile[:, ds(0, 128)],
                rhs=wtile[:],
                start=(i == 0),
                stop=(i == NWARM - 1),
            )

    # round 0: slice-0 projections in a dedicated psum scope (attention
    # pools don't exist yet, so four banks hold all head-tiles' groups
    # open across a k-split: the PE starts on the first half of x slice 0
    # while the second half is still in flight on the serial DMA device).
    with (
        tc.tile_pool(name="pj0a", bufs=1, space="PSUM") as pj0a,
        tc.tile_pool(name="pj0b", bufs=2, space="PSUM") as pj0b,
    ):

        def kq0(dst, wname, b_sb):
            pss = [
                pj0a.tile([128, QB1], F32, tag=f"p0{m}", name="ps_kq0")
                for m in range(N_DG)
            ]
            for kh in (0, 1):
                for m in range(N_DG):
                    for k in range(8 * kh, 8 * kh + 8):
                        nc.tensor.matmul(
                            pss[m][:],
                            lhsT=w_sbs[wname][:, ds(k * DGl + m * 128, 128)],
                            rhs=xa[:, ts(k, QB1)],
                            start=(k == 0),
                            stop=(k == NKD - 1),
                            skip_group_check=True,
                        )
            for m in range(N_DG):
                drain_pack_qk(dst, pss[m][:], b_sb[:, ds(m, 1)], m, 0)

        kq0(kt_res, "wk", bk_sb)
        for _ in gen_v(0, xa, pj0b):
            pass
        kq0(qt_res, "wq", bq_sb)

    ps_s = ctx.enter_context(tc.tile_pool(name="ps_s", bufs=2, space="PSUM"))
    ps_c = ctx.enter_context(tc.tile_pool(name="ps_c", bufs=2, space="PSUM"))
    ps_l = ctx.enter_context(tc.tile_pool(name="ps_l", bufs=1, space="PSUM"))

    with tc.tile_pool(name="pj", bufs=2, space="PSUM") as pj:
        # rounds 1-3: slice P woven with attention over q-block P-1
        for P in (1, 2, 3):
            xa = xb
            xb = load_x(P + 1) if P < 3 else None
            if P == 2:
                load_wo(0, 2)
                load_wo(2, 2)
            qb = P - 1
            units = [attn_unit(h, qb) for h in range(N_DG)]
            if P < 3:
                filler, fsteps = gen_slice(P, xa, pj), SLICE_STEPS
            else:
                # defer slice 3's v projection into round 4: its 13.6us of
                # PE work is only needed by the last PV tiles of q-block 3,
                # and round 4 is otherwise thin on PE filler relative to
                # its exp (ACT) load.
                def kq_only(s, x_sb):
                    yield from gen_kq(kt_res, "wk", bk_sb, s, x_sb, pj)
                    yield from gen_kq(qt_res, "wq", bq_sb, s, x_sb, pj)

                filler, fsteps = kq_only(P, xa), 2 * 4 * (NKD + 1)
                xa3 = xa
            weave(units, N_DG * ATTN_STEPS(qb), filler, fsteps)

    # round 4: last q-block's attention woven with output projection of
    # q-blocks 0-2; then the remaining output projection.
    with tc.tile_pool(name="pj3", bufs=3, space="PSUM") as pj3:
        units = [attn_unit(h, 3) for h in range(N_DG)]

        def p3_fill():
            yield from gen_v(3, xa3, pj3, tag="p3")
            for qb in range(3):
                yield from gen_p3(qb, pj3)

        weave(
            units,
            N_DG * ATTN_STEPS(3),
            p3_fill(),
            4 * (NKD + 1) + 3 * P3_STEPS,
        )
        for _ in gen_p3(3, pj3):
            pass


def build_program(enable_asserts=False):
    nc = bacc.Bacc(
        "TRN2",
        target_bir_lowering=False,
        debug=False,
        enable_asserts=enable_asserts,
        num_devices=N_CORES,
    )
    S, D, DGl = SEQ, D_MODEL, DG
    aps = {
        "xt": nc.dram_tensor("xt", [D, S], BF16, kind="ExternalInput").ap(),
        "wqt": nc.dram_tensor("wqt", [D, DGl], BF16, kind="ExternalInput").ap(),
        "wkt": nc.dram_tensor("wkt", [D, DGl], BF16, kind="ExternalInput").ap(),
        "wvt": nc.dram_tensor("wvt", [D, DGl], BF16, kind="ExternalInput").ap(),
        "wot": nc.dram_tensor("wot", [DGl, D], BF16, kind="ExternalInput").ap(),
        "bq": nc.dram_tensor("bq", [128, DGl // 128], F32, kind="ExternalInput").ap(),
        "bk": nc.dram_tensor("bk", [128, DGl // 128], F32, kind="ExternalInput").ap(),
        "bv": nc.dram_tensor("bv", [128, DGl], F32, kind="ExternalInput").ap(),
        "tri": nc.dram_tensor("tri", [128, 128], BF16, kind="ExternalInput").ap(),
        "ones": nc.dram_tensor("ones", [128, 1], BF16, kind="ExternalInput").ap(),
        "out": nc.dram_tensor("out", [S, D], F32, kind="ExternalOutput").ap(),
    }
    with tile.TileContext(nc) as tc:
        with ExitStack() as ctx:
            _mha_body(ctx, tc, aps)
    nc.compile()
    return nc


def make_tri():
    """Multiplicative causal mask for the 128x128 diagonal block: 1 where
    kpos<=qpos (keep), 0 where masked (applied to exp'd scores)."""
    p = np.arange(128)[:, None]
    f = np.arange(128)[None, :]
    return np.where(p <= f, 1.0, 0.0).astype(BFNP)


def shard_inputs(x, wq, bq, wk, bk, wv, bv, wo, bo):
    """Build the 8 per-core input maps (host-side layout prep + bf16)."""
    tri = make_tri()
    xts = [
        np.ascontiguousarray(np.asarray(x[b], np.float32).T).astype(BFNP)
        for b in range(BATCH)
    ]
    in_maps = []
    for c in range(N_CORES):
        b, g = divmod(c, N_GROUPS)
        sl = slice(g * DG, (g + 1) * DG)
        in_maps.append(
            {
                "xt": xts[b],
                "wqt": np.ascontiguousarray(np.asarray(wq, np.float32)[sl].T).astype(BFNP),
                "wkt": np.ascontiguousarray(np.asarray(wk, np.float32)[sl].T).astype(BFNP),
                "wvt": np.ascontiguousarray(np.asarray(wv, np.float32)[sl].T).astype(BFNP),
                "wot": np.ascontiguousarray(np.asarray(wo, np.float32)[:, sl].T).astype(BFNP),
                "bq": np.ascontiguousarray(
                    np.asarray(bq, np.float32)[sl].reshape(-1, 128).T
                ),
                "bk": np.ascontiguousarray(
                    np.asarray(bk, np.float32)[sl].reshape(-1, 128).T
                ),
                "bv": np.ascontiguousarray(
                    np.broadcast_to(np.asarray(bv, np.float32)[sl], (128, DG))
                ),
                "tri": tri,
                "ones": np.ones((128, 1), BFNP),
            }
        )
    return in_maps


_NC_CACHE = {}


def get_program():
    if "nc" not in _NC_CACHE:
        _NC_CACHE["nc"] = build_program()
    return _NC_CACHE["nc"]


def run_sharded(inputs, trace=False):
    nc = get_program()
    in_maps = shard_inputs(**inputs)
    res = run_bass_kernel_spmd(nc, in_maps, list(range(N_CORES)), trace=trace)
    bo = np.asarray(inputs["bo"], np.float32)
    full = np.empty((BATCH, SEQ, D_MODEL), np.float32)
    for b in range(BATCH):
        acc = res.results[b * N_GROUPS]["out"].copy()
        for g in range(1, N_GROUPS):
            acc += res.results[b * N_GROUPS + g]["out"]
        full[b] = acc + bo
    return full, res


def kernel(**inputs):
    out, _ = run_sharded(inputs, trace=False)
    return out
